# revision 10
# baseline (speedup 1.0000x reference)
"""Trainium2 Bass kernel for 16-head causal attention (B=2, L=2048, D=1024).

Sharding (8 NeuronCores, one chip):
  - Head tensor-parallel: core c computes heads {2c, 2c+1} for BOTH batches.
    QKV projections are computed in transposed (d-major) layout directly:
    Q^T = Wq_pair @ x^T (x^T and W^T are host-prepped), so the S^T = K^T-x-Q
    matmul needs no on-device activation transposes.
  - Attention: S^T tiles [128 kv, 512 q], additive causal mask on diagonal
    blocks, un-stabilized exp on ScalarE (scale=1/8 folded into ACT), P*V with
    a ones-augmented V (softmax denominators fall out of the matmul), fp32r.
  - One 8-way AllToAll redistributes ctx^T from head-sharded to
    (batch, L/4-slice)-sharded; each core then computes
    out^T = Wo @ ctx_full^T for its 512-row output slice.
  - Host: transpose/concat per-core out^T slices into the full output.

All matmuls run as float32r (FP22 multiply, fp32 accumulate): ~3e-4
scale-relative output error vs the fp32 reference.
"""

import numpy as np

import concourse.bass as bass
import concourse.mybir as mybir
import concourse.tile as tile
from concourse import bacc
from concourse.bass_utils import run_bass_kernel_spmd

F32 = mybir.dt.float32
F32R = mybir.dt.float32r

B, L, D = 2, 2048, 1024
NCORES = 8
QC = 512   # q-chunk width
KB = 128   # kv-block width
NEG = -3.0e4  # additive mask; exp(scale*NEG) == 0 exactly in fp32

_CACHE: dict = {}


def _build_program(n_iters: int = 1, debug: bool = False):
    """Build the SPMD Bass program (same on all cores; per-core data differs).

    n_iters > 1 repeats the whole compute body for wall-clock timing.
    debug=True enables named-tensor access for the simulator.
    """
    nc = bacc.Bacc(debug=debug)

    xT = nc.declare_dram_parameter("xT", [B, D, L], F32, isOutput=False)
    wqT = nc.declare_dram_parameter("wqT", [D, 128], F32, isOutput=False)
    wkT = nc.declare_dram_parameter("wkT", [D, 128], F32, isOutput=False)
    wvT = nc.declare_dram_parameter("wvT", [D, 128], F32, isOutput=False)
    woT = nc.declare_dram_parameter("woT", [D, D], F32, isOutput=False)
    maskadd = nc.declare_dram_parameter("maskadd", [4, 128, QC], F32, isOutput=False)
    outT = nc.declare_dram_parameter("outT", [D, QC], F32, isOutput=True)

    a2a_in = nc.dram_tensor("a2a_in", [NCORES * 128, QC], F32)
    a2a_out = nc.dram_tensor("a2a_out", [NCORES * 128, QC], F32)
    groups = [list(range(NCORES))]

    with tile.TileContext(nc) as tc:
        for _ in range(n_iters):
            _emit_iteration(nc, tc, xT, wqT, wkT, wvT, woT, maskadd, outT,
                            a2a_in, a2a_out, groups)

    nc.compile()
    return nc


def _emit_iteration(nc, tc, xT, wqT, wkT, wvT, woT, maskadd, outT,
                    a2a_in, a2a_out, groups):
    from concourse.masks import make_identity

    with (
        tc.tile_pool(name="const", bufs=1) as const_p,
        tc.tile_pool(name="w", bufs=1) as w_p,
        tc.tile_pool(name="qk", bufs=2) as qk_p,
        tc.tile_pool(name="vd", bufs=1) as vd_p,
        tc.tile_pool(name="vt", bufs=32) as v_p,
        tc.tile_pool(name="ctx", bufs=2) as ctx_p,
        tc.tile_pool(name="msk", bufs=4) as msk_p,
        tc.tile_pool(name="p", bufs=4) as p_p,
        tc.tile_pool(name="r", bufs=2) as r_p,
        tc.tile_pool(name="bs", bufs=2) as bs_p,
        tc.tile_pool(name="ps", bufs=2, space="PSUM") as ps_p,
        tc.tile_pool(name="st", bufs=4, space="PSUM") as st_p,
        tc.tile_pool(name="cps", bufs=2, space="PSUM") as cps_p,
    ):
        # ---- constants (built in F32: memset/affine_select lack f32r; DVE
        # copies convert) ----
        identity_f = const_p.tile([128, 128], F32)
        make_identity(nc, identity_f[:])
        identity = const_p.tile([128, 128], F32R)
        nc.vector.tensor_copy(identity[:], identity_f[:])
        ones_f = const_p.tile([128, 1], F32)
        nc.gpsimd.memset(ones_f[:], 1.0)
        ones64 = const_p.tile([1, 64], F32R)
        nc.vector.tensor_copy(ones64[:], ones_f[0:1, 0:1].to_broadcast([1, 64]))

        mask_sb = []
        for t in range(4):
            m = msk_p.tile([128, QC], F32, tag="mask")
            nc.sync.dma_start(m[:], maskadd[t])
            mask_sb.append(m)

        # ---- weights (QKV) ----
        w_sb = {}
        for name, par in (("wq", wqT), ("wk", wkT), ("wv", wvT)):
            w = w_p.tile([128, 8, 128], F32R, tag=name)
            nc.sync.dma_start(
                w[:], par.rearrange("(a p) m -> p a m", p=128).bitcast(F32R)
            )
            w_sb[name] = w

        ctx_sb = []  # per batch [128, 2048]
        with tc.tile_pool(name="xt", bufs=8) as xt_p:
            for b in range(B):
                # ---- load x^T for this batch ----
                xt = []
                for a in range(8):
                    x_tile = xt_p.tile([128, L], F32R, tag="xt")
                    nc.sync.dma_start(
                        x_tile[:], xT[b, 128 * a : 128 * (a + 1), :].bitcast(F32R)
                    )
                    xt.append(x_tile)

                # ---- projections (d-major): QT/KT/VdT [128, 2048] ----
                def proj(w, dst):
                    for n in range(4):
                        ps = ps_p.tile([128, QC], F32, tag="ps")
                        for a in range(8):
                            nc.tensor.matmul(
                                ps[:],
                                lhsT=w[:, a, :],
                                rhs=xt[a][:, QC * n : QC * (n + 1)],
                                start=(a == 0),
                                stop=(a == 7),
                            )
                        nc.vector.tensor_copy(dst[:, QC * n : QC * (n + 1)], ps[:])

                qT = qk_p.tile([128, L], F32R, tag="qT")
                kT = qk_p.tile([128, L], F32R, tag="kT")
                vdT = vd_p.tile([128, L], F32R, tag="vdT")
                proj(w_sb["wq"], qT)
                proj(w_sb["wk"], kT)
                proj(w_sb["wv"], vdT)

                # ---- V to kv-major via PE transpose; ones column appended ----
                v_tiles = []
                for kb in range(16):
                    tp = ps_p.tile([128, 128], F32R, tag="ps")
                    with nc.allow_low_precision(reason="pure transpose, no accumulation"):
                        nc.tensor.transpose(
                            tp[:], vdT[:, KB * kb : KB * (kb + 1)], identity[:]
                        )
                    vt = v_p.tile([128, 130], F32R, tag="vt")
                    for a in range(2):
                        nc.vector.tensor_copy(
                            vt[:, 65 * a : 65 * a + 64], tp[:, 64 * a : 64 * a + 64]
                        )
                        nc.vector.tensor_copy(
                            vt[:, 65 * a + 64 : 65 * a + 65], ones_f[:]
                        )
                    v_tiles.append(vt)

                # ---- attention ----
                ctxT = ctx_p.tile([128, L], F32, tag="ctxT")
                ctx_sb.append(ctxT)
                for jc in range(4):
                    qs = slice(QC * jc, QC * (jc + 1))
                    cps = [
                        cps_p.tile([65, QC], F32, tag="cps", name=f"cps{a}")
                        for a in range(2)
                    ]
                    nkb = 4 * jc + 4
                    for kb in range(nkb):
                        ks = slice(KB * kb, KB * (kb + 1))
                        p_t = []
                        for a in range(2):
                            da = slice(64 * a, 64 * (a + 1))
                            st = st_p.tile([128, QC], F32, tag="st")
                            nc.tensor.matmul(
                                st[:],
                                lhsT=kT[da, ks],
                                rhs=qT[da, qs],
                                start=True,
                                stop=True,
                                tile_position=(64 * a, 0),
                            )
                            if kb >= 4 * jc:
                                w = 128 * (kb - 4 * jc + 1)
                                nc.vector.tensor_add(
                                    st[:, :w], st[:, :w], mask_sb[kb - 4 * jc][:, :w]
                                )
                            p = p_p.tile([128, QC], F32R, tag="p")
                            nc.scalar.activation(
                                p[:], st[:], mybir.ActivationFunctionType.Exp,
                                scale=0.125,
                            )
                            p_t.append(p)
                        for a in range(2):
                            nc.tensor.matmul(
                                cps[a][:],
                                lhsT=v_tiles[kb][:, 65 * a : 65 * a + 65],
                                rhs=p_t[a][:],
                                start=(kb == 0),
                                stop=(kb == nkb - 1),
                            )
                    # normalize: ctxT rows for each head *= 1/denominator
                    for a in range(2):
                        r = r_p.tile([1, QC], F32R, tag="r")
                        with nc.allow_low_precision(
                            reason="f32r recip: 13 mantissa bits suffice"
                        ):
                            nc.vector.reciprocal(r[:], cps[a][64:65, :])
                        bc = ps_p.tile([64, QC], F32, tag="ps")
                        nc.tensor.matmul(
                            bc[:],
                            lhsT=ones64[:],
                            rhs=r[:],
                            start=True,
                            stop=True,
                        )
                        bs = bs_p.tile([64, QC], F32, tag="bs")
                        nc.vector.tensor_copy(bs[:], bc[:])
                        nc.vector.tensor_mul(
                            ctxT[64 * a : 64 * (a + 1), qs], cps[a][0:64, :], bs[:]
                        )

        # ---- A2A staging: one DMA per batch ----
        for b in range(B):
            nc.sync.dma_start(
                a2a_in[512 * b : 512 * (b + 1), :].rearrange(
                    "(a p) n -> p a n", p=128
                ),
                ctx_sb[b][:].rearrange("p (a n) -> p a n", a=4),
            )
        nc.gpsimd.collective_compute(
            "AllToAll",
            mybir.AluOpType.bypass,
            replica_groups=groups,
            ins=[a2a_in[:]],
            outs=[a2a_out[:]],
        )

        # ---- out projection: out^T = Wo @ ctx_full^T (for our q-slice) ----
        with (
            tc.tile_pool(name="wo", bufs=1) as wo_p,
            tc.tile_pool(name="ao", bufs=8) as ao_p,
            tc.tile_pool(name="os", bufs=4) as os_p,
        ):
            wo_sb = wo_p.tile([128, 8, D], F32R, tag="wo")
            nc.sync.dma_start(
                wo_sb[:], woT.rearrange("(a p) m -> p a m", p=128).bitcast(F32R)
            )
            ao = []
            for a in range(8):
                t = ao_p.tile([128, QC], F32R, tag="ao")
                nc.sync.dma_start(
                    t[:], a2a_out[128 * a : 128 * (a + 1), :].bitcast(F32R)
                )
                ao.append(t)
            for ob in range(8):
                ps = ps_p.tile([128, QC], F32, tag="ps")
                for a in range(8):
                    nc.tensor.matmul(
                        ps[:],
                        lhsT=wo_sb[:, a, 128 * ob : 128 * (ob + 1)],
                        rhs=ao[a][:],
                        start=(a == 0),
                        stop=(a == 7),
                    )
                o_sb = os_p.tile([128, QC], F32, tag="os")
                nc.vector.tensor_copy(o_sb[:], ps[:])
                nc.sync.dma_start(outT[128 * ob : 128 * (ob + 1), :], o_sb[:])


def _prep_in_maps(x, mask, Wq, Wk, Wv, Wo):
    x = np.ascontiguousarray(np.asarray(x, dtype=np.float32))
    Wq = np.asarray(Wq, dtype=np.float32)
    Wk = np.asarray(Wk, dtype=np.float32)
    Wv = np.asarray(Wv, dtype=np.float32)
    Wo = np.asarray(Wo, dtype=np.float32)
    mask = np.asarray(mask)

    xT = np.ascontiguousarray(x.transpose(0, 2, 1))  # [2, 1024, 2048]
    woT = np.ascontiguousarray(Wo.T)

    # additive causal-mask tiles for the 4 diagonal-block offsets
    m0 = mask[0]
    maskadd = np.stack(
        [
            np.where(m0[0:QC, KB * t : KB * (t + 1)].T, 0.0, NEG).astype(np.float32)
            for t in range(4)
        ]
    )

    in_maps = []
    for c in range(NCORES):
        rows = slice(128 * c, 128 * (c + 1))
        in_maps.append(
            {
                "xT": xT,
                "wqT": np.ascontiguousarray(Wq[rows, :].T),
                "wkT": np.ascontiguousarray(Wk[rows, :].T),
                "wvT": np.ascontiguousarray(Wv[rows, :].T),
                "woT": woT,
                "maskadd": maskadd,
            }
        )
    return in_maps


def _assemble(results):
    out = np.empty((B, L, D), np.float32)
    for c in range(NCORES):
        outT = results[c]["outT"]
        out[c // 4, QC * (c % 4) : QC * (c % 4 + 1), :] = outT.T
    return out


def get_program(n_iters: int = 1, debug: bool = False):
    key = ("prog", n_iters, debug)
    if key not in _CACHE:
        _CACHE[key] = _build_program(n_iters, debug=debug)
    return _CACHE[key]


def kernel(x, mask, Wq, Wk, Wv, Wo):
    nc = get_program()
    in_maps = _prep_in_maps(x, mask, Wq, Wk, Wv, Wo)
    res = run_bass_kernel_spmd(nc, in_maps, core_ids=list(range(NCORES)))
    return _assemble(res.results)
